# revision 1
# baseline (speedup 1.0000x reference)
"""Trainium2 Bass kernel for 3x3 conv (stride 1, pad 1) + bias.

x [32, 64, 224, 224] f32, weight [128, 64, 3, 3] f32, bias [128] f32
-> out [32, 128, 224, 224] f32.

Data-parallel over 8 NeuronCores: core c computes samples [4c, 4c+4).

Per-core scheme (v2, all dims hardcoded):
- x is zero-padded to [4, 64, 226, 226] on the host, so every strip DMA is
  fully contiguous and all matmul windows are uniform (no edge cases).
- float32r matmuls (1 cycle/row on PE, ~13-bit mantissa).
- K=128 packing: SBUF strip holds padded x rows on partitions 0-63 (top)
  and the same rows shifted one row down on partitions 64-127 (bottom,
  built by one SBUF->SBUF DMA per strip). One K=128 matmul computes the
  kh=0 AND kh=1 contributions together (weights for the two kh stacked on
  the partition halves); kh=2 is a K=64 top-half matmul.
  6 matmuls per 2-output-row block instead of 9.
- Strips of 56 output rows (58 padded input rows), double buffered.
  in/dup DMAs ride the ACT HWDGE ring, store DMAs the SP HWDGE ring, so
  input and output transfers overlap.
- PSUM accumulation; ScalarE evacuates psum->SBUF fused with the bias add;
  store tiles batch 8 output rows so each store DMA moves ~0.9 MB.
- weight is transposed/stacked and bias reshaped on host (numpy).
"""
import numpy as np

import concourse.bass as bass
import concourse.mybir as mybir
import concourse.tile as tile
from concourse import bacc
from concourse.bass_utils import run_bass_kernel_spmd
from concourse._compat import axon_active

N_CORES = 8
S = 4                 # samples per core
IC, OC, H, W = 64, 128, 224, 224
HP, WP = H + 2, W + 2  # padded input dims (226)
QROWS = 56            # output rows per strip
SROWS = QROWS + 2     # 58 padded input rows per strip
NQ = H // QROWS       # 4 strips per sample
BLK = 2               # output rows per block
OBLK = 8              # output rows per store tile (4 blocks)

F32R = mybir.dt.float32r
F32 = mybir.dt.float32


def build_module(repeat=1):
    nc = bacc.Bacc("TRN2", target_bir_lowering=False, debug=not axon_active(),
                   enable_asserts=True, num_devices=N_CORES)
    xs = nc.dram_tensor("xs", [S, IC, HP, WP], F32R, kind="ExternalInput").ap()
    # wpair[0:64, kw*128+oc] = w[oc, ic, kh=0, kw]; [64:128, ...] = kh=1
    wpair = nc.dram_tensor("wpair", [2 * IC, 3 * OC], F32R, kind="ExternalInput").ap()
    # wk2[ic, kw*128+oc] = w[oc, ic, kh=2, kw]
    wk2 = nc.dram_tensor("wk2", [IC, 3 * OC], F32R, kind="ExternalInput").ap()
    bias = nc.dram_tensor("bias", [OC, 1], F32, kind="ExternalInput").ap()
    out = nc.dram_tensor("out", [S, OC, H, W], F32, kind="ExternalOutput").ap()

    with tile.TileContext(nc) as tc:
        with tc.tile_pool(name="wp", bufs=1) as wp, \
             tc.tile_pool(name="xp", bufs=2) as xp, \
             tc.tile_pool(name="op", bufs=3) as op, \
             tc.tile_pool(name="pp", bufs=6, space="PSUM") as pp:
            wpt = wp.tile([2 * IC, 3 * OC], F32R)
            wk2t = wp.tile([IC, 3 * OC], F32R)
            btile = wp.tile([OC, 1], F32)
            nc.sync.dma_start(out=wpt, in_=wpair)
            nc.sync.dma_start(out=wk2t, in_=wk2)
            nc.sync.dma_start(out=btile, in_=bias)

            def compute():
                for s in range(S):
                    for q in range(NQ):
                        # strip covers padded rows 56q .. 56q+58
                        strip = xp.tile([2 * IC, SROWS * WP], F32R, tag="strip")
                        sr = strip.rearrange("p (r c) -> p r c", c=WP)
                        # top half: padded rows, fully contiguous both sides
                        nc.scalar.dma_start(
                            out=sr[0:IC, :, :],
                            in_=xs[s, :, q * QROWS:q * QROWS + SROWS, :])
                        # bottom half = top shifted one row-slot down
                        nc.scalar.dma_start(
                            out=strip[IC:2 * IC, 0:(SROWS - 1) * WP],
                            in_=strip[0:IC, WP:SROWS * WP])

                        for g in range(QROWS // OBLK):
                            ot = op.tile([OC, OBLK, W], F32)
                            for bb in range(OBLK // BLK):
                                u = g * OBLK + bb * BLK
                                oh = q * QROWS + u
                                psum = pp.tile([OC, BLK, W], F32)
                                # slot u holds padded row 56q+u = input row
                                # 56q+u-1; pair mm at slots (u, u+1):
                                #   top    -> rows oh-1, oh   (kh=0)
                                #   bottom -> rows oh,   oh+1 (kh=1)
                                for i, kw in enumerate((0, 1, 2)):
                                    rhs = sr[:, u:u + BLK, kw:kw + W]
                                    nc.tensor.matmul(
                                        psum, wpt[:, kw * OC:(kw + 1) * OC], rhs,
                                        start=(i == 0), stop=False,
                                        skip_group_check=True)
                                # kh=2: input rows oh+1, oh+2 = slots u+2, u+3
                                for i, kw in enumerate((0, 1, 2)):
                                    rhs = sr[0:IC, u + 2:u + 2 + BLK, kw:kw + W]
                                    nc.tensor.matmul(
                                        psum, wk2t[:, kw * OC:(kw + 1) * OC], rhs,
                                        start=False, stop=(i == 2),
                                        skip_group_check=True)
                                nc.scalar.activation(
                                    ot[:, bb * BLK:(bb + 1) * BLK, :].rearrange(
                                        "p a b -> p (a b)"),
                                    psum.rearrange("p a b -> p (a b)"),
                                    mybir.ActivationFunctionType.Identity,
                                    bias=btile)
                            oh0 = q * QROWS + g * OBLK
                            nc.sync.dma_start(out=out[s, :, oh0:oh0 + OBLK, :], in_=ot)

            if repeat == 1:
                compute()
            else:
                with tc.For_i(0, repeat, 1):
                    compute()

    nc.compile()
    return nc


def host_prep(weight, bias):
    w = np.asarray(weight, dtype=np.float32)          # [oc, ic, kh, kw]
    wt = np.transpose(w, (1, 3, 0, 2))                # [ic, kw, oc, kh]
    wpair = np.concatenate([wt[:, :, :, 0], wt[:, :, :, 1]], axis=0) \
        .reshape(2 * IC, 3 * OC)
    wk2 = np.ascontiguousarray(wt[:, :, :, 2]).reshape(IC, 3 * OC)
    b = np.asarray(bias, dtype=np.float32).reshape(OC, 1)
    return wpair, wk2, b


def pad_x(x):
    xp_ = np.zeros((x.shape[0], IC, HP, WP), np.float32)
    xp_[:, :, 1:1 + H, 1:1 + W] = x
    return xp_


_module_cache = {}


def get_module(repeat=1):
    if repeat not in _module_cache:
        _module_cache[repeat] = build_module(repeat)
    return _module_cache[repeat]


def kernel(x, weight, bias):
    x = np.asarray(x, dtype=np.float32)
    wpair, wk2, b = host_prep(weight, bias)
    xp_ = pad_x(x)
    nc = get_module()
    in_maps = [{"xs": xp_[c * S:(c + 1) * S], "wpair": wpair, "wk2": wk2,
                "bias": b} for c in range(N_CORES)]
    res = run_bass_kernel_spmd(nc, in_maps, core_ids=list(range(N_CORES)))
    return np.concatenate([res.results[c]["out"] for c in range(N_CORES)], axis=0)



# revision 2
# speedup vs baseline: 1.5812x; 1.5812x over previous
"""Trainium2 Bass kernel for 3x3 conv (stride 1, pad 1) + bias.

x [32, 64, 224, 224] f32, weight [128, 64, 3, 3] f32, bias [128] f32
-> out [32, 128, 224, 224] f32.

Data-parallel over 8 NeuronCores: core c computes samples [4c, 4c+4).

Per-core scheme (v3, all dims hardcoded):
- Inputs cast to bf16 on host (PSUM accumulation stays fp32; rel err
  ~1.5e-3, well inside the gate). bf16 also enables FWL fast weight load.
- The K=128 row-pair packing is built in HBM on the host: xdup holds the
  zero-padded rows on partitions 0-63 and the same rows shifted one row
  up on partitions 64-127. One contiguous 128-partition DMA per strip
  replaces the old load + SBUF->SBUF shift copy (halves fabric traffic,
  uses all 16 SDMA engines).
- One K=128 matmul computes the kh=0 AND kh=1 contributions together
  (weights for the two kh stacked on the partition halves); kh=2 is a
  K=64 top-half matmul. 6 matmuls per 2-output-row block.
- Strips of 56 output rows (58 padded input rows), triple buffered.
  Strip t+1's load is emitted BEFORE strip t's activations so the ACT
  HWDGE ring dispatches it while strip t computes (the v2 kernel
  serialized load -> compute per strip). Store DMAs ride the SP ring.
- PSUM accumulation; ScalarE evacuates psum->SBUF fused with the bias
  add; store tiles batch 8 output rows (~0.9 MB per store DMA).
"""
import numpy as np
import ml_dtypes

import concourse.bass as bass
import concourse.mybir as mybir
import concourse.tile as tile
from concourse import bacc
from concourse.bass_utils import run_bass_kernel_spmd
from concourse._compat import axon_active

N_CORES = 8
S = 4                 # samples per core
IC, OC, H, W = 64, 128, 224, 224
HP, WP = H + 2, W + 2  # padded input dims (226)
QROWS = 56            # output rows per strip
SROWS = QROWS + 2     # 58 padded input rows per strip
NQ = H // QROWS       # 4 strips per sample
BLK = 2               # output rows per block
OBLK = 8              # output rows per store tile (4 blocks)

BF16 = mybir.dt.bfloat16
F32 = mybir.dt.float32
NPBF16 = ml_dtypes.bfloat16


def build_module(repeat=1):
    nc = bacc.Bacc("TRN2", target_bir_lowering=False, debug=not axon_active(),
                   enable_asserts=True, num_devices=N_CORES)
    # xdup[s, 0:64, r, c]   = Ppad[ic, r, c]    (zero-padded input rows)
    # xdup[s, 64:128, r, c] = Ppad[ic, r+1, c]  (shifted one row up)
    xdup = nc.dram_tensor("xdup", [S, 2 * IC, HP, WP], BF16,
                          kind="ExternalInput").ap()
    # wpair[0:64, kw*128+oc] = w[oc, ic, kh=0, kw]; [64:128, ...] = kh=1
    wpair = nc.dram_tensor("wpair", [2 * IC, 3 * OC], BF16,
                           kind="ExternalInput").ap()
    # wk2[ic, kw*128+oc] = w[oc, ic, kh=2, kw]
    wk2 = nc.dram_tensor("wk2", [IC, 3 * OC], BF16, kind="ExternalInput").ap()
    bias = nc.dram_tensor("bias", [OC, 1], F32, kind="ExternalInput").ap()
    out = nc.dram_tensor("out", [S, OC, H, W], F32, kind="ExternalOutput").ap()

    NT = S * NQ  # 16 strips, flattened (s, q)

    with tile.TileContext(nc) as tc:
        with tc.tile_pool(name="wp", bufs=1) as wp, \
             tc.tile_pool(name="xp", bufs=3) as xp, \
             tc.tile_pool(name="op", bufs=3) as op, \
             tc.tile_pool(name="pp", bufs=6, space="PSUM") as pp:
            wpt = wp.tile([2 * IC, 3 * OC], BF16)
            wk2t = wp.tile([IC, 3 * OC], BF16)
            btile = wp.tile([OC, 1], F32)
            nc.sync.dma_start(out=wpt, in_=wpair)
            nc.sync.dma_start(out=wk2t, in_=wk2)
            nc.sync.dma_start(out=btile, in_=bias)

            def load_strip(t):
                s, q = divmod(t, NQ)
                strip = xp.tile([2 * IC, SROWS * WP], BF16, tag="strip")
                sr = strip.rearrange("p (r c) -> p r c", c=WP)
                nc.scalar.dma_start(
                    out=sr,
                    in_=xdup[s, :, q * QROWS:q * QROWS + SROWS, :])
                return sr

            def compute():
                strips = [load_strip(0)]
                for t in range(NT):
                    s, q = divmod(t, NQ)
                    if t + 1 < NT:
                        strips.append(load_strip(t + 1))
                    sr = strips[t]

                    for g in range(QROWS // OBLK):
                        ot = op.tile([OC, OBLK, W], F32)
                        for bb in range(OBLK // BLK):
                            u = g * OBLK + bb * BLK
                            oh = q * QROWS + u
                            psum = pp.tile([OC, BLK, W], F32)
                            # slot u holds padded row 56q+u; pair mm at
                            # slots (u, u+1):
                            #   top    -> rows oh, oh+1  (kh=0)
                            #   bottom -> rows oh+1,oh+2 (kh=1)
                            for i, kw in enumerate((0, 1, 2)):
                                rhs = sr[:, u:u + BLK, kw:kw + W]
                                nc.tensor.matmul(
                                    psum, wpt[:, kw * OC:(kw + 1) * OC], rhs,
                                    start=(i == 0), stop=False,
                                    skip_group_check=True)
                            # kh=2: padded rows oh+2, oh+3 = top slots u+2, u+3
                            for i, kw in enumerate((0, 1, 2)):
                                rhs = sr[0:IC, u + 2:u + 2 + BLK, kw:kw + W]
                                nc.tensor.matmul(
                                    psum, wk2t[:, kw * OC:(kw + 1) * OC], rhs,
                                    start=False, stop=(i == 2),
                                    skip_group_check=True)
                            nc.scalar.activation(
                                ot[:, bb * BLK:(bb + 1) * BLK, :].rearrange(
                                    "p a b -> p (a b)"),
                                psum.rearrange("p a b -> p (a b)"),
                                mybir.ActivationFunctionType.Identity,
                                bias=btile)
                        oh0 = q * QROWS + g * OBLK
                        nc.sync.dma_start(out=out[s, :, oh0:oh0 + OBLK, :],
                                          in_=ot)

            if repeat == 1:
                compute()
            else:
                with tc.For_i(0, repeat, 1):
                    compute()

    nc.compile()
    return nc


def host_prep(weight, bias):
    w = np.asarray(weight, dtype=np.float32)          # [oc, ic, kh, kw]
    wt = np.transpose(w, (1, 3, 0, 2))                # [ic, kw, oc, kh]
    wpair = np.concatenate([wt[:, :, :, 0], wt[:, :, :, 1]], axis=0) \
        .reshape(2 * IC, 3 * OC).astype(NPBF16)
    wk2 = np.ascontiguousarray(wt[:, :, :, 2]).reshape(IC, 3 * OC) \
        .astype(NPBF16)
    b = np.asarray(bias, dtype=np.float32).reshape(OC, 1)
    return wpair, wk2, b


def pad_x(x):
    """[N, 64, 224, 224] f32 -> [N, 128, 226, 226] bf16 row-pair dup."""
    n = x.shape[0]
    xb = np.asarray(x, dtype=np.float32).astype(NPBF16)
    xd = np.zeros((n, 2 * IC, HP, WP), NPBF16)
    xd[:, 0:IC, 1:1 + H, 1:1 + W] = xb          # Ppad[ic, r, c]
    xd[:, IC:2 * IC, 0:H, 1:1 + W] = xb         # Ppad[ic, r+1, c]
    return xd


_module_cache = {}


def get_module(repeat=1):
    if repeat not in _module_cache:
        _module_cache[repeat] = build_module(repeat)
    return _module_cache[repeat]


def kernel(x, weight, bias):
    wpair, wk2, b = host_prep(weight, bias)
    xd = pad_x(x)
    nc = get_module()
    in_maps = [{"xdup": xd[c * S:(c + 1) * S], "wpair": wpair, "wk2": wk2,
                "bias": b} for c in range(N_CORES)]
    res = run_bass_kernel_spmd(nc, in_maps, core_ids=list(range(N_CORES)))
    return np.concatenate([res.results[c]["out"] for c in range(N_CORES)],
                          axis=0)


# revision 7
# speedup vs baseline: 1.8477x; 1.1686x over previous
"""Trainium2 Bass kernel for 3x3 conv (stride 1, pad 1) + bias.

x [32, 64, 224, 224] f32, weight [128, 64, 3, 3] f32, bias [128] f32
-> out [32, 128, 224, 224] f32.

Data-parallel over 8 NeuronCores: core c computes samples [4c, 4c+4).

Per-core scheme (v3, all dims hardcoded):
- Inputs cast to bf16 on host (PSUM accumulation stays fp32; rel err
  ~1.5e-3, well inside the gate). bf16 also enables FWL fast weight load.
- The K=128 row-pair packing is built in HBM on the host: xdup holds the
  zero-padded rows on partitions 0-63 and the same rows shifted one row
  up on partitions 64-127. One contiguous 128-partition DMA per strip
  replaces the old load + SBUF->SBUF shift copy (halves fabric traffic,
  uses all 16 SDMA engines).
- One K=128 matmul computes the kh=0 AND kh=1 contributions together
  (weights for the two kh stacked on the partition halves); kh=2 is a
  K=64 top-half matmul. 6 matmuls per 2-output-row block.
- Strips of 56 output rows (58 padded input rows), triple buffered.
  Strip t+1's load is emitted BEFORE strip t's activations so the ACT
  HWDGE ring dispatches it while strip t computes (the v2 kernel
  serialized load -> compute per strip). Store DMAs ride the SP ring.
- PSUM accumulation; ScalarE evacuates psum->SBUF fused with the bias
  add, writing bf16 (halves store traffic; host upcasts to f32). Store
  tiles batch 8 output rows (~0.46 MB per store DMA).
"""
import numpy as np
import ml_dtypes

import concourse.bass as bass
import concourse.mybir as mybir
import concourse.tile as tile
from concourse import bacc
from concourse.bass_utils import run_bass_kernel_spmd
from concourse._compat import axon_active

N_CORES = 8
S = 4                 # samples per core
IC, OC, H, W = 64, 128, 224, 224
HP, WP = H + 2, W + 2  # padded input dims (226)
QROWS = 56            # output rows per strip
SROWS = QROWS + 2     # 58 padded input rows per strip
NQ = H // QROWS       # 4 strips per sample
BLK = 2               # output rows per block
OBLK = 8              # output rows per store tile (4 blocks)

BF16 = mybir.dt.bfloat16
F32 = mybir.dt.float32
NPBF16 = ml_dtypes.bfloat16


def build_module(repeat=1):
    nc = bacc.Bacc("TRN2", target_bir_lowering=False, debug=not axon_active(),
                   enable_asserts=True, num_devices=N_CORES)
    # xdup[s, 0:64, r, c]   = Ppad[ic, r, c]    (zero-padded input rows)
    # xdup[s, 64:128, r, c] = Ppad[ic, r+1, c]  (shifted one row up)
    xdup = nc.dram_tensor("xdup", [S, 2 * IC, HP, WP], BF16,
                          kind="ExternalInput").ap()
    # wpair[0:64, kw*128+oc] = w[oc, ic, kh=0, kw]; [64:128, ...] = kh=1
    wpair = nc.dram_tensor("wpair", [2 * IC, 3 * OC], BF16,
                           kind="ExternalInput").ap()
    # wk2[ic, kw*128+oc] = w[oc, ic, kh=2, kw]
    wk2 = nc.dram_tensor("wk2", [IC, 3 * OC], BF16, kind="ExternalInput").ap()
    bias = nc.dram_tensor("bias", [OC, 1], F32, kind="ExternalInput").ap()
    out = nc.dram_tensor("out", [S, OC, H, W], BF16, kind="ExternalOutput").ap()

    NT = S * NQ  # 16 strips, flattened (s, q)

    with tile.TileContext(nc) as tc:
        with tc.tile_pool(name="wp", bufs=1) as wp, \
             tc.tile_pool(name="xp", bufs=3) as xp, \
             tc.tile_pool(name="op", bufs=3) as op, \
             tc.tile_pool(name="pp", bufs=2, space="PSUM") as pp:
            wpt = wp.tile([2 * IC, 3 * OC], BF16)
            wk2t = wp.tile([IC, 3 * OC], BF16)
            btile = wp.tile([OC, 1], F32)
            nc.sync.dma_start(out=wpt, in_=wpair)
            nc.sync.dma_start(out=wk2t, in_=wk2)
            nc.sync.dma_start(out=btile, in_=bias)

            def load_strip(t):
                s, q = divmod(t, NQ)
                strip = xp.tile([2 * IC, SROWS * WP], BF16, tag="strip")
                sr = strip.rearrange("p (r c) -> p r c", c=WP)
                nc.scalar.dma_start(
                    out=sr,
                    in_=xdup[s, :, q * QROWS:q * QROWS + SROWS, :])
                return sr

            def compute():
                strips = [load_strip(0)]
                for t in range(NT):
                    s, q = divmod(t, NQ)
                    if t + 1 < NT:
                        strips.append(load_strip(t + 1))
                    sr = strips[t]

                    # taps: 3 K=128 row-pair (kh0+kh1) + 3 K=64 (kh2)
                    for g in range(QROWS // OBLK):
                        ot = op.tile([OC, OBLK, W], BF16)
                        psums = [pp.tile([OC, BLK, W], F32, name="ps%d" % bb)
                                 for bb in range(OBLK // BLK)]
                        # tap-major: 4 consecutive matmuls share each
                        # stationary weight slice (one per psum bank).
                        for ti in range(6):
                            kw = ti % 3
                            wtile = wpt if ti < 3 else wk2t
                            lhsT = wtile[:, kw * OC:(kw + 1) * OC]
                            for bb in range(OBLK // BLK):
                                u = g * OBLK + bb * BLK
                                if ti < 3:
                                    # top -> rows oh,oh+1 (kh=0);
                                    # bottom -> rows oh+1,oh+2 (kh=1)
                                    rhs = sr[:, u:u + BLK, kw:kw + W]
                                else:
                                    # kh=2: padded rows oh+2,oh+3 = top
                                    # slots u+2,u+3
                                    rhs = sr[0:IC, u + 2:u + 2 + BLK,
                                             kw:kw + W]
                                nc.tensor.matmul(
                                    psums[bb], lhsT, rhs,
                                    start=(ti == 0), stop=(ti == 5),
                                    skip_group_check=True)
                        for bb in range(OBLK // BLK):
                            nc.scalar.activation(
                                ot[:, bb * BLK:(bb + 1) * BLK, :].rearrange(
                                    "p a b -> p (a b)"),
                                psums[bb].rearrange("p a b -> p (a b)"),
                                mybir.ActivationFunctionType.Identity,
                                bias=btile)
                        oh0 = q * QROWS + g * OBLK
                        nc.sync.dma_start(out=out[s, :, oh0:oh0 + OBLK, :],
                                          in_=ot)

            if repeat == 1:
                compute()
            else:
                with tc.For_i(0, repeat, 1):
                    compute()

    nc.compile()
    return nc


def host_prep(weight, bias):
    w = np.asarray(weight, dtype=np.float32)          # [oc, ic, kh, kw]
    wt = np.transpose(w, (1, 3, 0, 2))                # [ic, kw, oc, kh]
    wpair = np.concatenate([wt[:, :, :, 0], wt[:, :, :, 1]], axis=0) \
        .reshape(2 * IC, 3 * OC).astype(NPBF16)
    wk2 = np.ascontiguousarray(wt[:, :, :, 2]).reshape(IC, 3 * OC) \
        .astype(NPBF16)
    b = np.asarray(bias, dtype=np.float32).reshape(OC, 1)
    return wpair, wk2, b


def pad_x(x):
    """[N, 64, 224, 224] f32 -> [N, 128, 226, 226] bf16 row-pair dup."""
    n = x.shape[0]
    xb = np.asarray(x, dtype=np.float32).astype(NPBF16)
    xd = np.zeros((n, 2 * IC, HP, WP), NPBF16)
    xd[:, 0:IC, 1:1 + H, 1:1 + W] = xb          # Ppad[ic, r, c]
    xd[:, IC:2 * IC, 0:H, 1:1 + W] = xb         # Ppad[ic, r+1, c]
    return xd


_module_cache = {}


def get_module(repeat=1):
    if repeat not in _module_cache:
        _module_cache[repeat] = build_module(repeat)
    return _module_cache[repeat]


def kernel(x, weight, bias):
    wpair, wk2, b = host_prep(weight, bias)
    xd = pad_x(x)
    nc = get_module()
    in_maps = [{"xdup": xd[c * S:(c + 1) * S], "wpair": wpair, "wk2": wk2,
                "bias": b} for c in range(N_CORES)]
    res = run_bass_kernel_spmd(nc, in_maps, core_ids=list(range(N_CORES)))
    return np.concatenate([res.results[c]["out"] for c in range(N_CORES)],
                          axis=0).astype(np.float32)
